# revision 24
# baseline (speedup 1.0000x reference)
"""CustomBatchNorm2D forward on 8 Trainium2 NeuronCores.

Reference (per channel j over the full batch):
    mean[j] = mean(x[:, j, :, :])
    t[i,j]  = sum_hw x[i,j,:,:]                (raw per-sample channel sums)
    diag[j] = sum_i (t[i,j] - HW*mean[j])^2 / HW
    out     = gamma[j]*abs(diag[j])*(x - mean[j]) + beta[j]

Algebraic form used here (T = sum_i t[i,j], Q = sum_i t[i,j]^2):
    |diag[j]| = |T[j]^2/N - Q[j]| / HW
    out       = A[j]*x + B[j],  A = gamma*|diag|,  B = beta - A*T/(N*HW)

Sharding: over channels C (512 -> 64 per core). Each core owns the full
batch for its 64 channels, so all statistics are computed locally and no
collective is needed. Per core: 16 tiles of [128, 1024] f32, one per
sample pair (partition p = parity*64 + channel, free = h*w; sample
s = 2*pair + parity). Channel stats need a fold of partition p with
p+64, done with two tiny SBUF->SBUF DMAs on separate HWDGE rings.

Schedule: x loads stream on the sync ring while DVE (even pairs) and ACT
(odd pairs, Copy+accum_out) compute per-sample sums; T/Q totals land in
one [128,2] tile, folded with 2 parallel DMAs; short stats chain with A
(DVE) and B (ACT) computed concurrently; in-place normalize alternates
DVE/ACT per pair and stores stream back on the sync ring.
"""

import numpy as np

import concourse.bacc as bacc
import concourse.mybir as mybir
import concourse.tile as tile
from concourse.bass_utils import run_bass_kernel_spmd

N, C, H, W = 32, 512, 32, 32
NCORES = 8
CPC = C // NCORES          # 64 channels per core
HW = H * W                 # 1024
NPAIR = N // 2             # 16 sample pairs = 16 tiles per core
f32 = mybir.dt.float32

_CACHE = {}


def _build(reps=1, fold="pe", accum="act", rings="one"):
    key = (reps, fold, accum, rings)
    if key in _CACHE:
        return _CACHE[key]

    nc = bacc.Bacc(
        "TRN2",
        target_bir_lowering=False,
        debug=False,
        enable_asserts=False,
        num_devices=NCORES,
    )
    x = nc.dram_tensor("x", [N, CPC, H, W], f32, kind="ExternalInput")
    gamma = nc.dram_tensor("gamma", [CPC], f32, kind="ExternalInput")
    beta = nc.dram_tensor("beta", [CPC], f32, kind="ExternalInput")
    out = nc.dram_tensor("out", [N, CPC, H, W], f32, kind="ExternalOutput")

    # [16 pairs, 128 partitions = (parity, channel), 1024 hw]
    xr = x[:].rearrange("(np pr) c h w -> np (pr c) (h w)", pr=2)
    outr = out[:].rearrange("(np pr) c h w -> np (pr c) (h w)", pr=2)

    AX = mybir.AxisListType.X
    MUL = mybir.AluOpType.mult
    ADD = mybir.AluOpType.add
    SUB = mybir.AluOpType.subtract
    AF = mybir.ActivationFunctionType

    with tile.TileContext(nc) as tc:
        with (
            tc.tile_pool(name="data", bufs=1) as dp,
            tc.tile_pool(name="stats", bufs=1) as sp,
            tc.tile_pool(name="psum", bufs=1, space="PSUM") as pp,
        ):
          for _rep in range(reps):
            t_all = sp.tile([128, NPAIR], f32, name="t_all", tag="t_all")
            g = sp.tile([128, 1], f32, name="g", tag="g")
            b = sp.tile([128, 1], f32, name="b", tag="b")

            if fold == "pe":
                # fold matrix M2[p,f] = 1.0 if p == f (mod 64): M2.T @ v
                # adds partition p and p^64, total lands in both halves
                w_i = sp.tile([128, 128], mybir.dt.int32, name="w_i", tag="w_i")
                wm = sp.tile([128, 128], mybir.dt.int32, name="wm", tag="wm")
                M2 = sp.tile([128, 128], f32, name="M2", tag="M2")
                nc.gpsimd.iota(w_i, pattern=[[-1, 128]], base=128, channel_multiplier=1)
                nc.vector.tensor_scalar(wm, w_i, 63, None, mybir.AluOpType.bitwise_and)
                nc.vector.tensor_scalar(M2, wm, 0, None, mybir.AluOpType.is_equal)

            # issue every load first (gen-only on the queues), evens on
            # the sync ring, odds on the scalar ring when rings=="split";
            # the last pair is split in two chunks so its tail-critical
            # reduction overlaps the second chunk's DMA
            xtiles = []
            half = HW // 2
            for p in range(NPAIR):
                xt = dp.tile([128, HW], f32, name=f"xt{p}", tag=f"xt{p}")
                xtiles.append(xt)
                if p == NPAIR - 1:
                    nc.sync.dma_start(xt[:, 0:half], xr[p][:, 0:half])
                    nc.sync.dma_start(xt[:, half:HW], xr[p][:, half:HW])
                elif p % 2 == 0 or rings != "split":
                    nc.sync.dma_start(xt, xr[p])
                else:
                    nc.scalar.dma_start(xt, xr[p])
                if p == 1:
                    # gamma/beta duplicated into both partition halves
                    nc.scalar.dma_start(g[0:64, :], gamma[:, None])
                    nc.scalar.dma_start(g[64:128, :], gamma[:, None])
                    nc.scalar.dma_start(b[0:64, :], beta[:, None])
                    nc.scalar.dma_start(b[64:128, :], beta[:, None])

            # per-pair reductions: DVE for even pairs and the split last
            # pair; ACT (Copy + accum_out) for odd pairs and pair 14 so
            # DVE is free for the tail-critical chunks
            for p in range(NPAIR):
                xt = xtiles[p]
                if p == NPAIR - 1:
                    tx = sp.tile([128, 1], f32, name="tx", tag="tx")
                    nc.vector.reduce_sum(
                        t_all[:, p : p + 1], xt[:, 0:half], axis=AX
                    )
                    nc.vector.reduce_sum(tx, xt[:, half:HW], axis=AX)
                    nc.vector.tensor_add(
                        t_all[:, p : p + 1], t_all[:, p : p + 1], tx
                    )
                elif (p % 2 == 0 and p != NPAIR - 2) or accum not in ("both", "act"):
                    nc.vector.reduce_sum(t_all[:, p : p + 1], xt, axis=AX)
                else:
                    scr = sp.tile([128, HW], f32, name="scr", tag="scr")
                    nc.scalar.activation(
                        scr, xt, AF.Copy, accum_out=t_all[:, p : p + 1]
                    )

            # T (col 0) and Q (col 1) totals over the 16 pair columns
            TQ = sp.tile([128, 2], f32, name="TQ", tag="TQ")
            sq16 = sp.tile([128, NPAIR], f32, name="sq16", tag="sq16")
            nc.vector.reduce_sum(TQ[:, 0:1], t_all[:, :], axis=AX)
            # NOTE: tensor_tensor_reduce's accum_out crashes at runtime on
            # this stack (NEFF INTERNAL error), so square+reduce explicitly
            nc.vector.tensor_mul(sq16, t_all[:, :], t_all[:, :])
            nc.vector.reduce_sum(TQ[:, 1:2], sq16[:, :], axis=AX)

            TQf = sp.tile([128, 2], f32, name="TQf", tag="TQf")
            if fold == "pe":
                # fold partition halves on the (idle) tensor engine; PSUM
                # can feed only one input per op, so copy to SBUF once
                TQp = pp.tile([128, 2], f32, name="TQp", tag="TQp")
                nc.tensor.matmul(TQp, M2, TQ, start=True, stop=True)
                nc.vector.tensor_copy(TQf, TQp)
            else:
                # fold via two tiny SBUF->SBUF DMAs on separate rings
                TQs = sp.tile([128, 2], f32, name="TQs", tag="TQs")
                nc.sync.dma_start(TQs[0:64, :], TQ[64:128, :])
                nc.scalar.dma_start(TQs[64:128, :], TQ[0:64, :])
                nc.vector.tensor_add(TQf, TQ, TQs)
            T = TQf[:, 0:1]
            Q = TQf[:, 1:2]

            # A = gamma*|T^2/N - Q|/HW ; B = beta + |..|*gamma*(-T/(N*HW))
            mneg = sp.tile([128, 1], f32, name="mneg", tag="mneg")
            gmneg = sp.tile([128, 1], f32, name="gmneg", tag="gmneg")
            sqT = sp.tile([128, 1], f32, name="sqT", tag="sqT")
            u = sp.tile([128, 1], f32, name="u", tag="u")
            au = sp.tile([128, 1], f32, name="au", tag="au")
            A = sp.tile([128, 1], f32, name="A", tag="A")
            B = sp.tile([128, 1], f32, name="B", tag="B")
            nc.vector.tensor_mul(sqT, T, T)
            nc.vector.scalar_tensor_tensor(u, sqT, 1.0 / N, Q, MUL, SUB)
            nc.vector.tensor_scalar_mul(mneg, T, -1.0 / (N * HW))
            nc.vector.tensor_mul(gmneg, g, mneg)
            nc.scalar.activation(au, u, AF.Abs, scale=1.0 / HW)
            nc.vector.tensor_mul(A, au, g)
            nc.scalar.activation(B, au, AF.Identity, bias=b[:, 0:1], scale=gmneg[:, 0:1])

            # normalize in place (split DVE/ACT) and store; pair 0 is split
            # in half so the store stream starts as early as possible
            for p in range(NPAIR):
                xt = xtiles[p]
                if p == 0:
                    half = HW // 2
                    for s in range(2):
                        sl = slice(s * half, (s + 1) * half)
                        nc.vector.tensor_scalar(
                            xt[:, sl], xt[:, sl], A[:, 0:1], B[:, 0:1], MUL, ADD
                        )
                        nc.sync.dma_start(outr[p][:, sl], xt[:, sl])
                    continue
                if p % 2 == 0:
                    nc.vector.tensor_scalar(
                        xt[:, :], xt[:, :], A[:, 0:1], B[:, 0:1], MUL, ADD
                    )
                else:
                    nc.scalar.activation(
                        xt[:, :], xt[:, :], AF.Identity,
                        bias=B[:, 0:1], scale=A[:, 0:1],
                    )
                steng = nc.scalar if (rings == "split" and p % 2 == 1) else nc.sync
                steng.dma_start(outr[p], xt)

    nc.compile()
    _CACHE[key] = nc
    return nc


def _in_maps(x, gamma, beta):
    x = np.ascontiguousarray(x, dtype=np.float32)
    gamma = np.ascontiguousarray(gamma, dtype=np.float32)
    beta = np.ascontiguousarray(beta, dtype=np.float32)
    maps = []
    for k in range(NCORES):
        sl = slice(k * CPC, (k + 1) * CPC)
        maps.append(
            {
                "x": np.ascontiguousarray(x[:, sl]),
                "gamma": np.ascontiguousarray(gamma[sl]),
                "beta": np.ascontiguousarray(beta[sl]),
            }
        )
    return maps


def run(x, gamma, beta, trace=False, **kw):
    """Run on hardware; returns (full_output, BassKernelResults)."""
    nc = _build()
    res = run_bass_kernel_spmd(
        nc, _in_maps(x, gamma, beta), list(range(NCORES)), trace=trace, **kw
    )
    out = np.concatenate([res.results[k]["out"] for k in range(NCORES)], axis=1)
    return out, res


def kernel(x, gamma, beta):
    out, _ = run(x, gamma, beta)
    return out


# revision 32
# speedup vs baseline: 1.0163x; 1.0163x over previous
"""CustomBatchNorm2D forward on 8 Trainium2 NeuronCores.

Reference (per channel j over the full batch):
    mean[j] = mean(x[:, j, :, :])
    t[i,j]  = sum_hw x[i,j,:,:]                (raw per-sample channel sums)
    diag[j] = sum_i (t[i,j] - HW*mean[j])^2 / HW
    out     = gamma[j]*abs(diag[j])*(x - mean[j]) + beta[j]

Algebraic form used here (T = sum_i t[i,j], Q = sum_i t[i,j]^2):
    |diag[j]| = |T[j]^2/N - Q[j]| / HW
    out       = A[j]*x + B[j],  A = gamma*|diag|,  B = beta - A*T/(N*HW)

Sharding: over channels C (512 -> 64 per core). Each core owns the full
batch for its 64 channels, so all statistics are computed locally and no
collective is needed. Per core: 16 tiles of [128, 1024] f32, one per
sample pair (partition p = parity*64 + channel, free = h*w; sample
s = 2*pair + parity). Channel stats need a fold of partition p with
p+64, done with two tiny SBUF->SBUF DMAs on separate HWDGE rings.

Schedule: x loads stream on the sync ring while DVE (even pairs) and ACT
(odd pairs, Copy+accum_out) compute per-sample sums; T/Q totals land in
one [128,2] tile, folded with 2 parallel DMAs; short stats chain with A
(DVE) and B (ACT) computed concurrently; in-place normalize alternates
DVE/ACT per pair and stores stream back on the sync ring.
"""

import numpy as np

import concourse.bacc as bacc
import concourse.mybir as mybir
import concourse.tile as tile
from concourse.bass_utils import run_bass_kernel_spmd

N, C, H, W = 32, 512, 32, 32
NCORES = 8
CPC = C // NCORES          # 64 channels per core
HW = H * W                 # 1024
NPAIR = N // 2             # 16 sample pairs = 16 tiles per core
f32 = mybir.dt.float32

_CACHE = {}


def _build(reps=1, fold="pe", accum="act", rings="one"):
    key = (reps, fold, accum, rings)
    if key in _CACHE:
        return _CACHE[key]

    nc = bacc.Bacc(
        "TRN2",
        target_bir_lowering=False,
        debug=False,
        enable_asserts=False,
        num_devices=NCORES,
    )
    x = nc.dram_tensor("x", [N, CPC, H, W], f32, kind="ExternalInput")
    gamma = nc.dram_tensor("gamma", [CPC], f32, kind="ExternalInput")
    beta = nc.dram_tensor("beta", [CPC], f32, kind="ExternalInput")
    out = nc.dram_tensor("out", [N, CPC, H, W], f32, kind="ExternalOutput")

    # [16 pairs, 128 partitions = (parity, channel), 1024 hw]
    xr = x[:].rearrange("(np pr) c h w -> np (pr c) (h w)", pr=2)
    outr = out[:].rearrange("(np pr) c h w -> np (pr c) (h w)", pr=2)

    AX = mybir.AxisListType.X
    MUL = mybir.AluOpType.mult
    ADD = mybir.AluOpType.add
    SUB = mybir.AluOpType.subtract
    AF = mybir.ActivationFunctionType

    with tile.TileContext(nc) as tc:
        with (
            tc.tile_pool(name="data", bufs=1) as dp,
            tc.tile_pool(name="stats", bufs=1) as sp,
            tc.tile_pool(name="psum", bufs=1, space="PSUM") as pp,
        ):
          for _rep in range(reps):
            t_all = sp.tile([128, NPAIR], f32, name="t_all", tag="t_all")
            g = sp.tile([128, 1], f32, name="g", tag="g")
            b = sp.tile([128, 1], f32, name="b", tag="b")

            if fold == "pe":
                # fold matrix M2[p,f] = 1.0 if p == f (mod 64): M2.T @ v
                # adds partition p and p^64, total lands in both halves
                w_i = sp.tile([128, 128], mybir.dt.int32, name="w_i", tag="w_i")
                wm = sp.tile([128, 128], mybir.dt.int32, name="wm", tag="wm")
                M2 = sp.tile([128, 128], f32, name="M2", tag="M2")
                nc.gpsimd.iota(w_i, pattern=[[-1, 128]], base=128, channel_multiplier=1)
                nc.vector.tensor_scalar(wm, w_i, 63, None, mybir.AluOpType.bitwise_and)
                nc.vector.tensor_scalar(M2, wm, 0, None, mybir.AluOpType.is_equal)

            # loads stream on the sync ring; each pair's reduction is
            # emitted right after its load (DVE for even pairs, ACT via
            # Copy+accum_out for odd pairs and pair 14 so DVE is free for
            # the tail-critical last pair, which is split in two chunks
            # so its reduction overlaps the second chunk's DMA)
            xtiles = []
            half = HW // 2
            for p in range(NPAIR):
                xt = dp.tile([128, HW], f32, name=f"xt{p}", tag=f"xt{p}")
                xtiles.append(xt)
                if p == NPAIR - 1:
                    tx = sp.tile([128, 1], f32, name="tx", tag="tx")
                    nc.sync.dma_start(xt[:, 0:half], xr[p][:, 0:half])
                    nc.sync.dma_start(xt[:, half:HW], xr[p][:, half:HW])
                    nc.vector.reduce_sum(
                        t_all[:, p : p + 1], xt[:, 0:half], axis=AX
                    )
                    nc.vector.reduce_sum(tx, xt[:, half:HW], axis=AX)
                    nc.vector.tensor_add(
                        t_all[:, p : p + 1], t_all[:, p : p + 1], tx
                    )
                elif (p % 2 == 0 and p != NPAIR - 2) or accum not in ("both", "act"):
                    nc.sync.dma_start(xt, xr[p])
                    nc.vector.reduce_sum(t_all[:, p : p + 1], xt, axis=AX)
                else:
                    nc.sync.dma_start(xt, xr[p])
                    scr = sp.tile([128, HW], f32, name="scr", tag="scr")
                    nc.scalar.activation(
                        scr, xt, AF.Copy, accum_out=t_all[:, p : p + 1]
                    )
                if p == 1:
                    # gamma/beta duplicated into both partition halves
                    nc.scalar.dma_start(g[0:64, :], gamma[:, None])
                    nc.scalar.dma_start(g[64:128, :], gamma[:, None])
                    nc.scalar.dma_start(b[0:64, :], beta[:, None])
                    nc.scalar.dma_start(b[64:128, :], beta[:, None])

            # T (col 0) and Q (col 1) totals over the 16 pair columns
            TQ = sp.tile([128, 2], f32, name="TQ", tag="TQ")
            sq16 = sp.tile([128, NPAIR], f32, name="sq16", tag="sq16")
            nc.vector.reduce_sum(TQ[:, 0:1], t_all[:, :], axis=AX)
            # NOTE: tensor_tensor_reduce's accum_out crashes at runtime on
            # this stack (NEFF INTERNAL error), so square+reduce explicitly
            nc.vector.tensor_mul(sq16, t_all[:, :], t_all[:, :])
            nc.vector.reduce_sum(TQ[:, 1:2], sq16[:, :], axis=AX)

            TQf = sp.tile([128, 2], f32, name="TQf", tag="TQf")
            if fold == "pe":
                # fold partition halves on the (idle) tensor engine; PSUM
                # can feed only one input per op, so copy to SBUF once
                TQp = pp.tile([128, 2], f32, name="TQp", tag="TQp")
                nc.tensor.matmul(TQp, M2, TQ, start=True, stop=True)
                nc.vector.tensor_copy(TQf, TQp)
            else:
                # fold via two tiny SBUF->SBUF DMAs on separate rings
                TQs = sp.tile([128, 2], f32, name="TQs", tag="TQs")
                nc.sync.dma_start(TQs[0:64, :], TQ[64:128, :])
                nc.scalar.dma_start(TQs[64:128, :], TQ[0:64, :])
                nc.vector.tensor_add(TQf, TQ, TQs)
            T = TQf[:, 0:1]
            Q = TQf[:, 1:2]

            # A = gamma*|T^2/N - Q|/HW ; B = beta + |..|*gamma*(-T/(N*HW))
            mneg = sp.tile([128, 1], f32, name="mneg", tag="mneg")
            gmneg = sp.tile([128, 1], f32, name="gmneg", tag="gmneg")
            sqT = sp.tile([128, 1], f32, name="sqT", tag="sqT")
            u = sp.tile([128, 1], f32, name="u", tag="u")
            au = sp.tile([128, 1], f32, name="au", tag="au")
            A = sp.tile([128, 1], f32, name="A", tag="A")
            B = sp.tile([128, 1], f32, name="B", tag="B")
            nc.vector.tensor_mul(sqT, T, T)
            nc.vector.scalar_tensor_tensor(u, sqT, 1.0 / N, Q, MUL, SUB)
            nc.vector.tensor_scalar_mul(mneg, T, -1.0 / (N * HW))
            nc.vector.tensor_mul(gmneg, g, mneg)
            nc.scalar.activation(au, u, AF.Abs, scale=1.0 / HW)
            nc.vector.tensor_mul(A, au, g)
            nc.scalar.activation(B, au, AF.Identity, bias=b[:, 0:1], scale=gmneg[:, 0:1])

            # normalize in place (split DVE/ACT) and store; pair 0 is split
            # in half so the store stream starts as early as possible
            for p in range(NPAIR):
                xt = xtiles[p]
                if p == 0:
                    half = HW // 2
                    for s in range(2):
                        sl = slice(s * half, (s + 1) * half)
                        nc.vector.tensor_scalar(
                            xt[:, sl], xt[:, sl], A[:, 0:1], B[:, 0:1], MUL, ADD
                        )
                        nc.sync.dma_start(outr[p][:, sl], xt[:, sl])
                    continue
                if p % 2 == 0:
                    nc.vector.tensor_scalar(
                        xt[:, :], xt[:, :], A[:, 0:1], B[:, 0:1], MUL, ADD
                    )
                else:
                    nc.scalar.activation(
                        xt[:, :], xt[:, :], AF.Identity,
                        bias=B[:, 0:1], scale=A[:, 0:1],
                    )
                steng = nc.scalar if (rings == "split" and p % 2 == 1) else nc.sync
                steng.dma_start(outr[p], xt)

    nc.compile()
    _CACHE[key] = nc
    return nc


def _in_maps(x, gamma, beta):
    x = np.ascontiguousarray(x, dtype=np.float32)
    gamma = np.ascontiguousarray(gamma, dtype=np.float32)
    beta = np.ascontiguousarray(beta, dtype=np.float32)
    maps = []
    for k in range(NCORES):
        sl = slice(k * CPC, (k + 1) * CPC)
        maps.append(
            {
                "x": np.ascontiguousarray(x[:, sl]),
                "gamma": np.ascontiguousarray(gamma[sl]),
                "beta": np.ascontiguousarray(beta[sl]),
            }
        )
    return maps


def run(x, gamma, beta, trace=False, **kw):
    """Run on hardware; returns (full_output, BassKernelResults)."""
    nc = _build()
    res = run_bass_kernel_spmd(
        nc, _in_maps(x, gamma, beta), list(range(NCORES)), trace=trace, **kw
    )
    out = np.concatenate([res.results[k]["out"] for k in range(NCORES)], axis=1)
    return out, res


def kernel(x, gamma, beta):
    out, _ = run(x, gamma, beta)
    return out


# revision 33
# speedup vs baseline: 1.2247x; 1.2051x over previous
"""CustomBatchNorm2D forward on 8 Trainium2 NeuronCores.

Reference (per channel j over the full batch):
    mean[j] = mean(x[:, j, :, :])
    t[i,j]  = sum_hw x[i,j,:,:]                (raw per-sample channel sums)
    diag[j] = sum_i (t[i,j] - HW*mean[j])^2 / HW
    out     = gamma[j]*abs(diag[j])*(x - mean[j]) + beta[j]

Algebraic form used here (T = sum_i t[i,j], Q = sum_i t[i,j]^2):
    |diag[j]| = |T[j]^2/N - Q[j]| / HW
    out       = A[j]*x + B[j],  A = gamma*|diag|,  B = beta - A*T/(N*HW)

Sharding: over channels C (512 -> 64 per core). Each core owns the full
batch for its 64 channels, so all statistics are computed locally and no
collective is needed.

Within a core the 64 channels are further split into TWO groups of 32 so
the load->stats->store serialization of one group hides under the DMA
stream of the other: the sync-ring FIFO runs [A loads][B loads][A
stores][B stores] back to back, group A's statistics compute while B is
still loading, and B's statistics finish long before the DMA pipe has
drained A's stores - the DMA engines never idle between the load and
store phases, so the kernel runs at the HBM roofline plus only fixed
startup/drain overhead.

The host-side shard copy (which kernel() needs anyway) pre-permutes each
core's input to [group, tile, 128, 1024] with partition p = quad*32 +
channel and sample i = 4*tile + quad, so every tile is one fully
contiguous 512 KB DMA. Channel totals then need a fold of partitions
p, p+32, p+64, p+96: done as one [128,128] matmul on the otherwise-idle
tensor engine against a mod-32 selection matrix built on-chip via iota.
The same matmul also broadcasts gamma/beta (loaded into quad-slot 0 of
the stats tile, other slots zeroed) to all four quad-slots. Per-sample
sums run on DVE (even tiles) and ACT via Copy+accum_out (odd tiles); the
in-place normalize alternates DVE/ACT the same way. Small stats tensors
are raw (non-pooled) SBUF allocations: tile-pool slot reuse for them
races with the x loads the scheduler hoists around them.
"""

import numpy as np

import concourse.bacc as bacc
import concourse.mybir as mybir
import concourse.tile as tile
from concourse.bass_utils import run_bass_kernel_spmd

N, C, H, W = 32, 512, 32, 32
NCORES = 8
CPC = C // NCORES          # 64 channels per core
HW = H * W                 # 1024
CG = 2                     # channel groups per core
CPG = CPC // CG            # 32 channels per group
SPT = 128 // CPG           # 4 samples per tile
NTG = N // SPT             # 8 tiles per group
f32 = mybir.dt.float32

_CACHE = {}


def _build(reps=1):
    if reps in _CACHE:
        return _CACHE[reps]

    nc = bacc.Bacc(
        "TRN2",
        target_bir_lowering=False,
        debug=False,
        enable_asserts=False,
        num_devices=NCORES,
    )
    x = nc.dram_tensor("x", [CG, NTG, 128, HW], f32, kind="ExternalInput")
    gamma = nc.dram_tensor("gamma", [CPC], f32, kind="ExternalInput")
    beta = nc.dram_tensor("beta", [CPC], f32, kind="ExternalInput")
    out = nc.dram_tensor("out", [CG, NTG, 128, HW], f32, kind="ExternalOutput")

    AX = mybir.AxisListType.X
    MUL = mybir.AluOpType.mult
    ADD = mybir.AluOpType.add
    SUB = mybir.AluOpType.subtract
    AF = mybir.ActivationFunctionType

    with tile.TileContext(nc) as tc:
        with (
            tc.tile_pool(name="data", bufs=1) as dp,
            tc.tile_pool(name="psum", bufs=1, space="PSUM") as pp,
        ):
          # fold matrix M4[p,f] = 1.0 if p == f (mod 32): M4.T @ v sums
          # the four quad-slots, leaving the total in all of them
          w_i = nc.alloc_sbuf_tensor("w_i", [128, 128], mybir.dt.int32).ap()
          M4 = nc.alloc_sbuf_tensor("M4", [128, 128], f32).ap()
          nc.gpsimd.iota(w_i, pattern=[[-1, 128]], base=128, channel_multiplier=1)
          nc.vector.tensor_scalar(w_i, w_i, CPG - 1, None, mybir.AluOpType.bitwise_and)
          nc.vector.tensor_scalar(M4, w_i, 0, None, mybir.AluOpType.is_equal)

          # small per-group stats tensors, raw-allocated, shared across reps
          stats_t = {}
          for g in range(CG):
            stats_t[g] = {
                name: nc.alloc_sbuf_tensor(f"{name}{g}", [128, w], f32).ap()
                for name, w in [
                    ("ST", 4), ("STf", 4), ("t", NTG), ("sq8", NTG),
                    ("mneg", 1), ("gmneg", 1), ("u", 1),
                    ("au", 1), ("A", 1), ("B", 1),
                ]
            }

          for _rep in range(reps):
            # every load up front so the sync-ring FIFO is
            # [A loads][B loads][A stores][B stores] with no idle slots
            xtiles = {}
            for g in range(CG):
                for q in range(NTG):
                    xt = dp.tile([128, HW], f32, name=f"x{g}_{q}", tag=f"x{g}_{q}")
                    nc.sync.dma_start(xt, x[g, q])
                    xtiles[g, q] = xt

            # stats tile cols: [T, Q, gamma, beta]; gamma/beta sit in
            # quad-slot 0 with the rest zeroed, so the fold matmul also
            # broadcasts them to all slots
            for g in range(CG):
                ST = stats_t[g]["ST"]
                nc.gpsimd.memset(ST[:, 2:4], 0.0)
                sl = slice(g * CPG, (g + 1) * CPG)
                nc.scalar.dma_start(ST[0:CPG, 2:3], gamma[sl][:, None])
                nc.scalar.dma_start(ST[0:CPG, 3:4], beta[sl][:, None])

            for g in range(CG):
                st = stats_t[g]
                # per-sample channel sums: DVE for even tiles, ACT
                # (Copy + accum_out) for odd tiles
                t_g = st["t"]
                for q in range(NTG):
                    xt = xtiles[g, q]
                    if q % 2 == 0:
                        nc.vector.reduce_sum(t_g[:, q : q + 1], xt, axis=AX)
                    else:
                        scr = dp.tile([128, HW], f32, name="scr", tag="scr")
                        nc.scalar.activation(
                            scr, xt, AF.Copy, accum_out=t_g[:, q : q + 1]
                        )

                # T (col 0) and Q (col 1) totals over the 8 tile columns
                # (squares are per-sample, before any cross-sample fold)
                ST = st["ST"]
                sq8 = st["sq8"]
                nc.vector.reduce_sum(ST[:, 0:1], t_g[:, :], axis=AX)
                nc.vector.tensor_mul(sq8, t_g[:, :], t_g[:, :])
                nc.vector.reduce_sum(ST[:, 1:2], sq8[:, :], axis=AX)

                # fold the four quad-slots on the tensor engine; PSUM can
                # feed only one input per op, so copy to SBUF once
                STp = pp.tile([128, 4], f32, name=f"STp{g}", tag=f"STp{g}")
                nc.tensor.matmul(STp, M4, ST, start=True, stop=True)
                STf = st["STf"]
                nc.vector.tensor_copy(STf, STp)
                T = STf[:, 0:1]
                Q = STf[:, 1:2]
                gt = STf[:, 2:3]
                bt = STf[:, 3:4]

                # A = gamma*|T^2/N - Q|/HW ; B = beta + |..|*gamma*(-T/(N*HW))
                mneg, gmneg = st["mneg"], st["gmneg"]
                u, au, A, B = st["u"], st["au"], st["A"], st["B"]
                nc.vector.tensor_scalar(u, T, T[:, 0:1], None, MUL)
                nc.vector.scalar_tensor_tensor(u, u, 1.0 / N, Q, MUL, SUB)
                nc.vector.tensor_scalar_mul(mneg, T, -1.0 / (N * HW))
                nc.vector.tensor_mul(gmneg, gt, mneg)
                nc.scalar.activation(au, u, AF.Abs, scale=1.0 / HW)
                nc.vector.tensor_mul(A, au, gt)
                nc.scalar.activation(
                    B, au, AF.Identity, bias=bt[:, 0:1], scale=gmneg[:, 0:1]
                )

                # normalize in place (split DVE/ACT) and store
                for q in range(NTG):
                    xt = xtiles[g, q]
                    if q % 2 == 0:
                        nc.vector.tensor_scalar(
                            xt[:, :], xt[:, :], A[:, 0:1], B[:, 0:1], MUL, ADD
                        )
                    else:
                        nc.scalar.activation(
                            xt[:, :], xt[:, :], AF.Identity,
                            bias=B[:, 0:1], scale=A[:, 0:1],
                        )
                    nc.sync.dma_start(out[g, q], xt)

    nc.compile()
    _CACHE[reps] = nc
    return nc


def _in_maps(x, gamma, beta):
    x = np.ascontiguousarray(x, dtype=np.float32)
    gamma = np.ascontiguousarray(gamma, dtype=np.float32)
    beta = np.ascontiguousarray(beta, dtype=np.float32)
    maps = []
    for k in range(NCORES):
        sl = slice(k * CPC, (k + 1) * CPC)
        # [N, CPC, H, W] -> [CG, NTG, SPT*CPG=128, HW] with sample
        # i = SPT*tile + quad and channel j = CG_group*CPG + c
        xk = x[:, sl].reshape(NTG, SPT, CG, CPG, HW)
        xk = np.ascontiguousarray(xk.transpose(2, 0, 1, 3, 4)).reshape(
            CG, NTG, 128, HW
        )
        maps.append(
            {
                "x": xk,
                "gamma": np.ascontiguousarray(gamma[sl]),
                "beta": np.ascontiguousarray(beta[sl]),
            }
        )
    return maps


def _unshard(res):
    outs = []
    for k in range(NCORES):
        ok = res.results[k]["out"].reshape(CG, NTG, SPT, CPG, HW)
        ok = ok.transpose(1, 2, 0, 3, 4).reshape(N, CPC, H, W)
        outs.append(ok)
    return np.ascontiguousarray(np.concatenate(outs, axis=1))


def run(x, gamma, beta, trace=False, **kw):
    """Run on hardware; returns (full_output, BassKernelResults)."""
    nc = _build()
    res = run_bass_kernel_spmd(
        nc, _in_maps(x, gamma, beta), list(range(NCORES)), trace=trace, **kw
    )
    return _unshard(res), res


def kernel(x, gamma, beta):
    out, _ = run(x, gamma, beta)
    return out
